# revision 5
# baseline (speedup 1.0000x reference)
"""Grouped Conv2d (512 groups, 2->2 ch/group, 3x3 VALID) on 8 trn2 NeuronCores.

Strategy (v2 — 1-D Winograd F(2,3) along W):
  - Shard the 512 groups across 8 cores: 64 groups = 128 channels per core.
  - Winograd F(2,3) on the W axis only: the even/odd output columns share
    four transform-domain planes m_a[oc, i, q] (q = output column pair),
    computed as 12 accumulating PE matmuls per image (4 Winograd coords x
    3 kh taps, kh accumulated in PSUM) over 54x27 moving elements each —
    a 33% PE-cycle cut vs the direct 9-tap decomposition.
  - fp16 everywhere off-PSUM: halves DMA traffic and enables the DVE
    2x packed mode for the column transforms.
  - Host pre-splits x into even/odd column planes so every DVE transform
    op reads/writes stride-1 fp16; host re-interleaves the two output
    parity planes at the end.
  - Per 18-row chunk: 12 matmuls -> 4 PSUM banks, one ACT copy evicts all
    4 banks to fp16 SBUF, DVE does the 4-op inverse transform
    (ye = m0+m1+m2, yo = m1-m2-m3) into parity planes DMA'd per image.
"""

import sys

import numpy as np

for _p in ("/opt/trn_rl_repo",):
    if _p not in sys.path:
        sys.path.insert(0, _p)

import concourse.bacc as bacc
import concourse.bass as bass
import concourse.tile as tile
from concourse import mybir
from concourse.bass_utils import run_bass_kernel_spmd

N_CORES = 8
B, C, H, W = 16, 1024, 56, 56
KH = KW = 3
HO, WO = H - KH + 1, W - KW + 1  # 54, 54
CPC = C // N_CORES  # 128 channels (64 groups) per core
Q = WO // 2  # 27 output column pairs
NA = 4  # winograd F(2,3) coords
RC = 18  # output rows per PSUM chunk (18*27 = 486 <= 512 fp32 bank)
NCH = HO // RC  # 3 chunks
EW = W // 2  # 28 even/odd input columns

_NC_CACHE = {}


def _build_program():
    nc = bacc.Bacc(
        "TRN2", target_bir_lowering=False, debug=False, num_devices=N_CORES
    )
    f16 = mybir.dt.float16
    f32 = mybir.dt.float32

    xe_d = nc.declare_dram_parameter("xe", [B, CPC, H, EW], f16, isOutput=False)
    xo_d = nc.declare_dram_parameter("xo", [B, CPC, H, EW], f16, isOutput=False)
    wm_d = nc.declare_dram_parameter(
        "wm", [CPC, NA * KH, CPC], f16, isOutput=False
    )
    ye_d = nc.declare_dram_parameter("ye", [B, CPC, HO * Q], f16, isOutput=True)
    yo_d = nc.declare_dram_parameter("yo", [B, CPC, HO * Q], f16, isOutput=True)

    with tile.TileContext(nc) as tc:
        with (
            tc.tile_pool(name="wpool", bufs=1) as wpool,
            tc.tile_pool(name="eopool", bufs=3) as eopool,
            tc.tile_pool(name="xtpool", bufs=2) as xtpool,
            tc.tile_pool(name="mpool", bufs=4) as mpool,
            tc.tile_pool(name="ypool", bufs=3) as ypool,
            tc.tile_pool(name="psum", bufs=2, space="PSUM") as ppool,
        ):
            wt = wpool.tile([CPC, NA * KH, CPC], f16)
            nc.sync.dma_start(out=wt[:], in_=wm_d[:])

            # Warm up the PE p-state clock ramp (low->mid->full after 3us of
            # continuous execution) before real work arrives; the first dummy
            # also absorbs the wt-DMA semaphore wait.
            ptw = ppool.tile([CPC, NA, 512], f32, tag="pt")
            for _ in range(18):
                nc.tensor.matmul(
                    ptw[:, 0, 0:256],
                    lhsT=wt[:, 0, :],
                    rhs=wt[:, 0:2, :],
                    start=True,
                    stop=True,
                )

            for n in range(B):
                eo = eopool.tile([CPC, 2, H, EW], f16)
                nc.sync.dma_start(out=eo[:, 0], in_=xe_d[n])
                nc.sync.dma_start(out=eo[:, 1], in_=xo_d[n])

                # Column transform: d = (E[q], O[q], E[q+1], O[q+1])
                #   xt0 = E[q]   - E[q+1]   (weights g0)
                #   xt1 = O[q]   + E[q+1]   (weights (g0+g1+g2)/2)
                #   xt2 = E[q+1] - O[q]     (weights (g0-g1+g2)/2)
                #   xt3 = O[q]   - O[q+1]   (weights g2)
                xt = xtpool.tile([CPC, NA, H, Q], f16)
                E0 = eo[:, 0, :, 0:Q]
                E1 = eo[:, 0, :, 1 : Q + 1]
                O0 = eo[:, 1, :, 0:Q]
                O1 = eo[:, 1, :, 1 : Q + 1]
                nc.vector.tensor_sub(xt[:, 0], E0, E1)
                nc.vector.tensor_add(xt[:, 1], O0, E1)
                nc.vector.tensor_sub(xt[:, 2], E1, O0)
                nc.vector.tensor_sub(xt[:, 3], O0, O1)

                yt = ypool.tile([CPC, 2, HO * Q], f16)
                for c in range(NCH):
                    r0 = c * RC
                    pt = ppool.tile([CPC, NA, 512], f32, tag="pt")
                    for a in range(NA):
                        for kh in range(KH):
                            nc.tensor.matmul(
                                pt[:, a, 0 : RC * Q],
                                lhsT=wt[:, a * KH + kh, :],
                                rhs=xt[:, a, r0 + kh : r0 + kh + RC, :],
                                start=(kh == 0),
                                stop=(kh == KH - 1),
                            )
                    # Evict all 4 banks in one ACT op (f32 PSUM -> f16 SBUF).
                    mt = mpool.tile([CPC, NA + 2, RC * Q], f16)
                    nc.scalar.activation(
                        mt[:, 0:NA],
                        pt[:, :, 0 : RC * Q],
                        mybir.ActivationFunctionType.Copy,
                    )
                    # Inverse transform (fp16, packed -> DVE 2x mode):
                    #   ye = m0 + (m1 + m2);  yo = (m1 - m2) - m3
                    s = mt[:, NA]
                    t = mt[:, NA + 1]
                    cs = slice(c * RC * Q, (c + 1) * RC * Q)
                    nc.gpsimd.tensor_sub(t, mt[:, 1], mt[:, 2])
                    nc.vector.tensor_add(s, mt[:, 1], mt[:, 2])
                    nc.vector.tensor_add(yt[:, 0, cs], mt[:, 0], s)
                    nc.vector.tensor_sub(yt[:, 1, cs], t, mt[:, 3])

                nc.sync.dma_start(out=ye_d[n], in_=yt[:, 0])
                nc.sync.dma_start(out=yo_d[n], in_=yt[:, 1])
    nc.compile()
    return nc


def _get_nc():
    if "nc" not in _NC_CACHE:
        _NC_CACHE["nc"] = _build_program()
    return _NC_CACHE["nc"]


def _make_wmats(w):
    """Per-core lhsT weight mats wm[ic, a*3+kh, oc], fp16.

    wt[oc, icg, kh, a]: Winograd G transform of w along kw:
      a=0: w0;  a=1: (w0+w1+w2)/2;  a=2: (w0-w1+w2)/2;  a=3: w2.
    """
    oc = np.arange(CPC)
    mats = []
    for cid in range(N_CORES):
        ws = np.asarray(w[cid * CPC : (cid + 1) * CPC], dtype=np.float64)
        ws = ws.reshape(CPC, 2, KH, KW)
        wt = np.empty((CPC, 2, KH, NA), dtype=np.float64)
        wt[..., 0] = ws[..., 0]
        wt[..., 1] = (ws[..., 0] + ws[..., 1] + ws[..., 2]) * 0.5
        wt[..., 2] = (ws[..., 0] - ws[..., 1] + ws[..., 2]) * 0.5
        wt[..., 3] = ws[..., 2]
        # wm[2g+icg, a*3+kh, oc=2g+j] = wt[oc, icg, kh, a]
        wm = np.zeros((CPC, NA * KH, CPC), dtype=np.float16)
        for icg in range(2):
            ic = (oc // 2) * 2 + icg
            # [oc, kh, a] -> [oc, a, kh] -> [oc, 12]
            tap = wt[oc, icg].transpose(0, 2, 1).reshape(CPC, NA * KH)
            wm[ic, :, oc] = tap.astype(np.float16)
        mats.append(wm)
    return mats


def _run(x, w, trace=False, **kwargs):
    nc = _get_nc()
    x = np.asarray(x, dtype=np.float32)
    wmats = _make_wmats(w)
    xe = x[:, :, :, 0::2].astype(np.float16)
    xo = x[:, :, :, 1::2].astype(np.float16)
    in_maps = [
        {
            "xe": np.ascontiguousarray(xe[:, cid * CPC : (cid + 1) * CPC]),
            "xo": np.ascontiguousarray(xo[:, cid * CPC : (cid + 1) * CPC]),
            "wm": wmats[cid],
        }
        for cid in range(N_CORES)
    ]
    res = run_bass_kernel_spmd(
        nc, in_maps, list(range(N_CORES)), trace=trace, **kwargs
    )
    y = np.empty((B, C, HO, WO), dtype=np.float32)
    for cid in range(N_CORES):
        sl = slice(cid * CPC, (cid + 1) * CPC)
        ye = np.asarray(res.results[cid]["ye"], dtype=np.float32)
        yo = np.asarray(res.results[cid]["yo"], dtype=np.float32)
        y[:, sl, :, 0::2] = ye.reshape(B, CPC, HO, Q)
        y[:, sl, :, 1::2] = yo.reshape(B, CPC, HO, Q)
    return y, res


def kernel(x, w):
    y, _ = _run(x, w, trace=False)
    return y
